# revision 2
# baseline (speedup 1.0000x reference)
"""Single-head causal attention on 8 TRN2 NeuronCores — v2 (PE-tiled).

Sharding: 2 cores per batch element (B=4); core parity p owns interleaved
128-row t-blocks {2j+p}. Host swaps adjacent xT column blocks on odd cores
so one SPMD program serves all cores (masks differ as data) — same scheme
as v1.

v2 changes (all verified on HW via microbenchmarks):
  - PE-array tiling: the 128x128 PE is 16 independent 32x32 subarrays.
    Matmuls placed at disjoint tile positions execute CONCURRENTLY.
  - Projections write PSUM column strips (tile_position (0,32c)) building
    kT4/vT4/qT4 "strip layouts" 4-way concurrently: strip c of partitions
    holds s-blocks == c (mod 4).
  - v[s,H] produced from vT via hardware XBAR DMA transposes (exact),
    removing v1's 128 latency-bound N=32 matmuls.
  - Flash scores: rounds of 2 s-blocks at different row strips
    (tile_position (32r,0)) -> 2-way concurrent; one exp ACT over
    [128, 2, 512-i0] with causal column trim.
  - attnV accumulators for the two active groups live at PSUM partition
    offsets 0/64 (tile_position (0,0)/(0,64)) -> 2-way concurrent,
    interleaved emission.
  - Output stored bf16; host casts to fp32 and folds bv@Wp + bp.
"""

import math
import sys

for _p in ("/opt/trn_rl_repo", "/opt/trn_rl_repo/concourse"):
    if _p not in sys.path:
        sys.path.insert(0, _p)

import ml_dtypes
import numpy as np

BF16 = ml_dtypes.bfloat16

B, T, D, H = 4, 4096, 512, 32
NSLOT = 16          # 128-row t-blocks per core
NSB = T // 128      # 32 s-blocks
NG = 4              # slot groups of 4 slots (512 t-cols)
SCALE = 1.0 / math.sqrt(32.0)

_CACHE = {}


def build_nc(debug=False):
    import concourse.mybir as mybir
    import concourse.tile as tile
    from concourse import bacc

    dt = mybir.dt
    nc = bacc.Bacc("TRN2", target_bir_lowering=False, debug=False)

    if debug:
        dbg_kT4 = nc.dram_tensor(
            "dbg_kT4", [128, 8, 128], dt.bfloat16, kind="ExternalOutput"
        ).ap()
        dbg_qT4 = nc.dram_tensor(
            "dbg_qT4", [128, NG, 512], dt.bfloat16, kind="ExternalOutput"
        ).ap()
        dbg_vones = nc.dram_tensor(
            "dbg_vones", [128, NSB, H + 1], dt.bfloat16, kind="ExternalOutput"
        ).ap()
        dbg_attnT0 = nc.dram_tensor(
            "dbg_attnT0", [33, 512], dt.bfloat16, kind="ExternalOutput"
        ).ap()

    xT = nc.dram_tensor("xT", [4, 128, T], dt.bfloat16, kind="ExternalInput").ap()
    # combined small operands: one DMA trigger each (~0.7us per trigger)
    wqkv = nc.dram_tensor(
        "wqkv", [3, 4, 128, H], dt.bfloat16, kind="ExternalInput"
    ).ap()
    wp4 = nc.dram_tensor("wp4", [128, D], dt.bfloat16, kind="ExternalInput").ap()
    bqk = nc.dram_tensor("bqk", [2, 128, 1], dt.float32, kind="ExternalInput").ap()
    maskid = nc.dram_tensor(
        "maskid", [128, 3, 128], dt.bfloat16, kind="ExternalInput"
    ).ap()
    out = nc.dram_tensor(
        "out", [NSLOT * 128, D], dt.bfloat16, kind="ExternalOutput"
    ).ap()

    with tile.TileContext(nc) as tc, tc.tile_pool(
        name="singles", bufs=1
    ) as singles, tc.tile_pool(name="exp_pool", bufs=8) as exp_pool, tc.tile_pool(
        name="attnT_pool", bufs=2
    ) as attnT_pool, tc.tile_pool(
        name="recip_pool", bufs=2
    ) as recip_pool, tc.tile_pool(name="out_pool", bufs=2) as out_pool:
        # ---- resident SBUF tensors -----------------------------------
        xT_sb = singles.tile([128, 4, T], dt.bfloat16)
        wqkv_sb = singles.tile([128, 3, 4, H], dt.bfloat16)
        wq_sb = wqkv_sb[:, 0, :, :]
        wk_sb = wqkv_sb[:, 1, :, :]
        wv_sb = wqkv_sb[:, 2, :, :]
        wp_sb = singles.tile([128, D], dt.bfloat16)
        bqk_sb = singles.tile([128, 2], dt.float32)
        bq_sb = bqk_sb[:, 0:1]
        bk_sb = bqk_sb[:, 1:2]
        maskid_sb = singles.tile([128, 3, 128], dt.bfloat16)
        maskA = maskid_sb[:, 0, :]
        maskB = maskid_sb[:, 1, :]
        ident_sb = maskid_sb[:, 2, 0:H]
        # strip layouts: partition 32c+h holds s-block 4*q8+c, col j
        kT4 = singles.tile([128, 8, 128], dt.bfloat16)
        vT4 = singles.tile([128, 8, 128], dt.bfloat16)
        # qT replicated on all 4 strips: partition 32r+h = q[t, h]
        qT4 = singles.tile([128, NG, 512], dt.bfloat16)
        vones = singles.tile([128, NSB, H + 1], dt.bfloat16)
        ones1 = singles.tile([128, 1], dt.bfloat16)

        # DMA order tuned for time-to-first-matmul: quarter 0 of x^T and
        # the qkv weights go first (split across the sync+scalar queues);
        # each trigger costs ~0.7us of queue time regardless of size, so
        # later quarters use half as many, bigger transfers.
        q0sl = slice(0, T // 4)
        nc.sync.dma_start(out=xT_sb[:, 0, q0sl], in_=xT[0, :, q0sl])
        nc.scalar.dma_start(out=xT_sb[:, 1, q0sl], in_=xT[1, :, q0sl])
        nc.sync.dma_start(out=xT_sb[:, 2, q0sl], in_=xT[2, :, q0sl])
        nc.scalar.dma_start(out=xT_sb[:, 3, q0sl], in_=xT[3, :, q0sl])
        nc.sync.dma_start(
            out=wqkv_sb, in_=wqkv.rearrange("w c p h -> p w c h")
        )
        nc.scalar.dma_start(out=bqk_sb, in_=bqk.rearrange("b p o -> p (b o)"))
        nc.sync.dma_start(out=maskid_sb, in_=maskid)
        for tq in range(1, 4):
            tsl = slice(tq * (T // 4), (tq + 1) * (T // 4))
            nc.sync.dma_start(
                out=xT_sb[:, 0:2, tsl],
                in_=xT[0:2, :, tsl].rearrange("e p t -> p e t"),
            )
            nc.scalar.dma_start(
                out=xT_sb[:, 2:4, tsl],
                in_=xT[2:4, :, tsl].rearrange("e p t -> p e t"),
            )
        nc.scalar.dma_start(out=wp_sb, in_=wp4)

        nc.vector.memset(vones, 1.0)
        nc.vector.memset(ones1, 1.0)

        # views: s = (4*q8 + strip)*128 + j ; own q cols at even positions
        xk = xT_sb.rearrange("p e (q f j) -> p e q f j", f=4, j=128)
        xq = xT_sb.rearrange("p e (g two b) -> p e g two b", two=2, b=128)

        # ---- projection emitters (column-strip tiled, 4-way) ---------
        def qT_chunk(ps_proj, tcq):
            qps = ps_proj.tile([128, 512], dt.float32, name="qps", tag="p")
            for e in range(4):
                for r in range(4):
                    nc.tensor.matmul(
                        qps[32 * r:32 * r + 32, :],
                        wq_sb[:, e, :],
                        xq[:, e, 4 * tcq:4 * tcq + 4, 0, :],
                        start=(e == 0),
                        stop=(e == 3),
                        skip_group_check=True,
                        tile_position=(0, 32 * r),
                    )
            nc.vector.tensor_scalar_add(qT4[:, tcq, :], qps, bq_sb)

        def k_quarter(ps_proj, tq):
            kps = ps_proj.tile([128, 256], dt.float32, name="kps", tag="p")
            for e in range(4):
                for c in range(4):
                    nc.tensor.matmul(
                        kps[32 * c:32 * c + 32, :],
                        wk_sb[:, e, :],
                        xk[:, e, 2 * tq:2 * tq + 2, c, :],
                        start=(e == 0),
                        stop=(e == 3),
                        skip_group_check=True,
                        tile_position=(0, 32 * c),
                    )
            nc.vector.tensor_scalar_add(
                kT4[:, 2 * tq:2 * tq + 2, :],
                kps.rearrange("p (f j) -> p f j", j=128),
                bk_sb,
            )

        def v_quarter(ps_proj, tq):
            vps = ps_proj.tile([128, 256], dt.float32, name="vps", tag="p")
            for e in range(4):
                for c in range(4):
                    nc.tensor.matmul(
                        vps[32 * c:32 * c + 32, :],
                        wv_sb[:, e, :],
                        xk[:, e, 2 * tq:2 * tq + 2, c, :],
                        start=(e == 0),
                        stop=(e == 3),
                        skip_group_check=True,
                        tile_position=(0, 32 * c),
                    )
            nc.vector.tensor_copy(
                vT4[:, 2 * tq:2 * tq + 2, :],
                vps.rearrange("p (f j) -> p f j", j=128),
            )

        def v_trans(ps_proj, sb):
            # v[s,H] = vT_slice.T via regular matmul against identity
            q8, c = divmod(sb, 4)
            tp = ps_proj.tile(
                [128, H], dt.float32, name="tp", tag="t", bufs=1
            )
            nc.tensor.matmul(
                tp,
                vT4[32 * c:32 * c + 32, q8, :],
                ident_sb[32 * c:32 * c + 32, :],
                start=True,
                stop=True,
                tile_position=(32 * c, 0),
            )
            nc.vector.tensor_copy(vones[:, sb, 0:H], tp)

        # ---- flash attention machinery -------------------------------
        def epi_slot(g, g_lo, i, acc, ps_out, ps_den):
            p64 = 64 * (g - g_lo)
            j = 4 * g + i
            tsl = slice(i * 128, (i + 1) * 128)
            attnT = attnT_pool.tile([128, 128], dt.bfloat16, name="attnT")
            nc.vector.tensor_copy(
                attnT[p64:p64 + 33, :], acc[p64:p64 + 33, tsl]
            )
            if debug and g == 0:
                nc.sync.dma_start(out=dbg_attnT0[:, tsl], in_=attnT[0:33, :])
            dps = ps_den.tile([128, 1], dt.float32, name="dps", tag="d")
            nc.tensor.matmul(
                dps,
                attnT[p64 + 32:p64 + 33, :],
                ones1[p64 + 32:p64 + 33, :],
                start=True,
                stop=True,
                tile_position=(p64 + 32, 0),
            )
            recip = recip_pool.tile([128, 1], dt.float32, name="recip")
            nc.vector.reciprocal(recip, dps)
            ops = ps_out.tile([128, D], dt.float32, name="ops", tag="o")
            nc.tensor.matmul(
                ops,
                attnT[p64:p64 + 32, :],
                wp_sb[p64:p64 + 32, :],
                start=True,
                stop=True,
                tile_position=(p64, 0),
            )
            osb = out_pool.tile([128, D], dt.bfloat16, name="osb")
            nc.vector.tensor_scalar_mul(osb, ops, recip)
            nc.sync.dma_start(out=out[j * 128:(j + 1) * 128, :], in_=osb)

        def epilogue(g, g_lo, acc, ps_out, ps_den):
            for i in range(4):
                epi_slot(g, g_lo, i, acc, ps_out, ps_den)

        def run_pass(ps_sc, acc, g_lo, g_hi, fillers, epi):
            def emit_items(items):
                # interleave the avs of up to two items (alternating groups
                # -> alternating PSUM column strips -> 2-way concurrency)
                for i in range(2):
                    for g, Rp, expt, i0 in items:
                        sb = 2 * Rp + i
                        p64 = 64 * (g - g_lo)
                        nc.tensor.matmul(
                            acc[p64:p64 + 33, i0:512],
                            vones[:, sb, :],
                            expt[:, i, i0:512],
                            start=(sb == 0),
                            stop=(sb == 8 * g + 7),
                            skip_group_check=True,
                            tile_position=(0, p64),
                        )
                if epi is not None:
                    for g, Rp, expt, i0 in items:
                        if Rp == 4 * g + 3:
                            epi(g)

            def emit_scores(g, R):
                band = R >= 4 * g
                i0 = 128 * (R - 4 * g) if band else 0
                scps = ps_sc.tile(
                    [128, 2, 512], dt.float32, name="scps", tag="sc"
                )
                for i in range(2):
                    sb = 2 * R + i
                    nc.tensor.matmul(
                        scps[:, i, i0:512],
                        kT4[32 * (sb % 4):32 * (sb % 4) + 32, sb // 4, :],
                        qT4[32 * (sb % 4):32 * (sb % 4) + 32, g, i0:512],
                        start=True,
                        stop=True,
                        tile_position=(32 * (sb % 4), 0),
                    )
                expt = exp_pool.tile(
                    [128, 2, 512], dt.bfloat16, name="expt", tag="e"
                )
                nc.scalar.activation(
                    expt[:, :, i0:512],
                    scps[:, :, i0:512],
                    mybir.ActivationFunctionType.Exp,
                    scale=SCALE,
                )
                if band:
                    nc.vector.tensor_mul(
                        expt[:, 0, i0:i0 + 128], expt[:, 0, i0:i0 + 128],
                        maskA,
                    )
                    nc.vector.tensor_mul(
                        expt[:, 1, i0:i0 + 128], expt[:, 1, i0:i0 + 128],
                        maskB,
                    )
                return (g, R, expt, i0)

            # Software-pipelined stream of (group, round) items. g_hi is
            # staggered one round behind g_lo so concurrently-issued scores
            # use disjoint PE row strips (true 4-way). attnVs trail by ~2
            # items; fillers (trailing projections) slot in between.
            pending = []
            nfill = 0
            for k in range(4 * g_hi + 5):
                for g, R in ((g_lo, k), (g_hi, k - 1)):
                    if R < 0 or R >= 4 * g + 4:
                        continue
                    pending.append(emit_scores(g, R))
                if fillers:
                    fillers.pop(0)()
                while len(pending) > 3:
                    n = 2 if len(pending) > 4 else 1
                    emit_items(pending[:n])
                    del pending[:n]
            while fillers:
                fillers.pop(0)()
            while pending:
                n = min(2, len(pending))
                emit_items(pending[:n])
                del pending[:n]

        # ---- schedule: pass(0,1) overlapped with trailing projections
        with tc.tile_pool(name="ps_acc", bufs=1, space="PSUM") as ps_acc:
            acc1 = ps_acc.tile([128, 512], dt.float32, name="acc", tag="acc")
            with tc.tile_pool(
                name="ps_sc1", bufs=2, space="PSUM"
            ) as ps_sc1, tc.tile_pool(
                name="ps_proj", bufs=2, space="PSUM"
            ) as ps_proj:
                # prologue: only what round 0 needs up front
                qT_chunk(ps_proj, 0)
                k_quarter(ps_proj, 0)
                v_quarter(ps_proj, 0)
                qT_chunk(ps_proj, 1)

                def vt_slice(lo, hi):
                    def f():
                        for sb in range(lo, hi):
                            v_trans(ps_proj, sb)
                    return f

                fillers = [
                    vt_slice(0, 4),
                    lambda: k_quarter(ps_proj, 1),
                    lambda: v_quarter(ps_proj, 1),
                    vt_slice(4, 8),
                    vt_slice(8, 12),
                    lambda: k_quarter(ps_proj, 2),
                    lambda: v_quarter(ps_proj, 2),
                    vt_slice(12, 16),
                    lambda: qT_chunk(ps_proj, 2),
                    lambda: k_quarter(ps_proj, 3),
                    lambda: v_quarter(ps_proj, 3),
                    lambda: qT_chunk(ps_proj, 3),
                    vt_slice(16, 24),
                    vt_slice(24, 32),
                ]
                run_pass(ps_sc1, acc1, 0, 1, fillers, epi=None)

            with tc.tile_pool(
                name="ps_sc2", bufs=2, space="PSUM"
            ) as ps_sc2, tc.tile_pool(
                name="ps_out", bufs=2, space="PSUM"
            ) as ps_out, tc.tile_pool(
                name="ps_den", bufs=1, space="PSUM"
            ) as ps_den:
                epilogue(0, 0, acc1, ps_out, ps_den)
                epilogue(1, 0, acc1, ps_out, ps_den)
                acc2 = ps_acc.tile(
                    [128, 512], dt.float32, name="acc", tag="acc"
                )
                run_pass(
                    ps_sc2, acc2, 2, 3, [],
                    epi=lambda g: epilogue(g, 2, acc2, ps_out, ps_den),
                )

        if debug:
            nc.sync.dma_start(out=dbg_kT4, in_=kT4)
            nc.sync.dma_start(out=dbg_qT4, in_=qT4)
            nc.sync.dma_start(out=dbg_vones, in_=vones)

    nc.compile()
    return nc


def _get_nc():
    if "nc" not in _CACHE:
        _CACHE["nc"] = build_nc()
    return _CACHE["nc"]


def make_in_maps(x, Wq, bq, Wk, bk, Wv, bv, Wp, bp):
    """Build the 8 per-core input maps (host-side sharding)."""
    x = np.asarray(x, dtype=np.float32)
    tri = np.tril(np.ones((128, 128), dtype=np.float32)).T  # [s,t]: 1 iff s<=t
    wqkv_s = np.ascontiguousarray(
        np.stack(
            [np.asarray(w, np.float32).reshape(4, 128, H) for w in (Wq, Wk, Wv)]
        )
    ).astype(BF16)
    wp_s = np.ascontiguousarray(
        np.tile(np.asarray(Wp, np.float32), (4, 1))
    ).astype(BF16)
    bqk_s = np.ascontiguousarray(
        np.stack(
            [
                np.tile(np.asarray(b, np.float32), 4).reshape(128, 1)
                for b in (bq, bk)
            ]
        )
    )
    ident = np.zeros((128, 128), np.float32)
    ident[:, 0:H] = np.tile(np.eye(H, dtype=np.float32), (4, 1))

    in_maps = []
    for c in range(8):
        b, p = divmod(c, 2)
        xb = x[b]  # [T, D]
        if p == 1:
            xb = xb.reshape(T // 256, 2, 128, D)[:, ::-1].reshape(T, D)
        xT_c = np.ascontiguousarray(xb.T).astype(BF16).reshape(4, 128, T)
        if p == 0:
            m = np.stack([tri, np.zeros((128, 128), np.float32)])
        else:
            m = np.stack([tri, np.ones((128, 128), np.float32)])
        maskid = np.ascontiguousarray(
            np.stack([m[0], m[1], ident], axis=1)
        )  # [128, 3, 128]
        in_maps.append(
            {
                "xT": xT_c,
                "wqkv": wqkv_s,
                "wp4": wp_s,
                "bqk": bqk_s,
                "maskid": maskid.astype(BF16),
            }
        )
    return in_maps


def assemble_out(results, bv, Wp, bp):
    """Gather per-core [2048, 512] bf16 outputs into [B, T, D] fp32."""
    out = np.empty((B, T, D), dtype=np.float32)
    for c in range(8):
        b, p = divmod(c, 2)
        oc = np.asarray(results[c]["out"]).astype(np.float32).reshape(
            NSLOT, 128, D
        )
        for j in range(NSLOT):
            g = 2 * j + p
            out[b, g * 128:(g + 1) * 128, :] = oc[j]
    out += (
        np.asarray(bv, np.float32) @ np.asarray(Wp, np.float32)
        + np.asarray(bp, np.float32)
    )[None, None, :]
    return out


def run_axon_percore(nc, in_maps, n_cores=8):
    """Run the same single-core NEFF on n_cores axon devices.

    bass2jax.run_bass_via_pjrt's multi-core branch uses shard_map over
    an 8-device mesh; under the axon loopback relay that execution
    never completes. The kernel is pure data-parallel (no collectives),
    so n_cores independent per-device jit calls are semantically
    identical; jax's async dispatch lets them run concurrently.
    """
    import jax
    import concourse.mybir as mybir
    from concourse import bass2jax

    bass2jax.install_neuronx_cc_hook()

    partition_name = (
        nc.partition_id_tensor.name if nc.partition_id_tensor else None
    )
    in_names = []
    out_names = []
    out_avals = []
    zero_outs = []
    for alloc in nc.m.functions[0].allocations:
        if not isinstance(alloc, mybir.MemoryLocationSet):
            continue
        name = alloc.memorylocations[0].name
        if alloc.kind == "ExternalInput":
            if name != partition_name:
                in_names.append(name)
        elif alloc.kind == "ExternalOutput":
            out_names.append(name)
            shape = tuple(alloc.tensor_shape)
            dtype = mybir.dt.np(alloc.dtype)
            out_avals.append(jax.core.ShapedArray(shape, dtype))
            zero_outs.append(np.zeros(shape, dtype))
    n_params = len(in_names)
    all_names = in_names + out_names
    if partition_name is not None:
        all_names = all_names + [partition_name]

    def _body(*args):
        operands = list(args)
        if partition_name is not None:
            operands.append(bass2jax.partition_id_tensor())
        outs = bass2jax._bass_exec_p.bind(
            *operands,
            out_avals=tuple(out_avals),
            in_names=tuple(all_names),
            out_names=tuple(out_names),
            lowering_input_output_aliases=(),
            sim_require_finite=True,
            sim_require_nnan=True,
            nc=nc,
        )
        return tuple(outs)

    donate = tuple(range(n_params, n_params + len(out_names)))
    f = jax.jit(_body, donate_argnums=donate, keep_unused=True)
    devices = jax.devices()[:n_cores]
    pending = []
    for c in range(n_cores):
        args = [
            jax.device_put(np.asarray(in_maps[c][k]), devices[c])
            for k in in_names
        ] + [jax.device_put(z, devices[c]) for z in zero_outs]
        pending.append(f(*args))
    return [
        {name: np.asarray(outs[i]) for i, name in enumerate(out_names)}
        for outs in pending
    ]


def kernel(x, Wq, bq, Wk, bk, Wv, bv, Wp, bp):
    from concourse import bass_utils
    from concourse._compat import axon_active

    nc = _get_nc()
    in_maps = make_in_maps(x, Wq, bq, Wk, bk, Wv, bv, Wp, bp)
    if axon_active():
        results = run_axon_percore(nc, in_maps)
    else:
        res = bass_utils.run_bass_kernel_spmd(
            nc, in_maps, core_ids=list(range(8))
        )
        results = res.results
    return assemble_out(results, bv, Wp, bp)
